# revision 6
# baseline (speedup 1.0000x reference)
"""MultiHeadAttention Trainium2 kernel (8 NeuronCores, Bass/Tile).

Problem: B=2, S=2048, D=1024, H=16, DK=64 fp32 MHA (torch-Linear style
projections, softmax attention, output projection).

Sharding: core c = (batch b = c//4, head-group g = c%4); each core handles
4 heads of one batch. Per-core kernel computes, entirely in a transposed
layout (features on partitions, sequence on the free axis):
  qhT/khT  = (Wg x^T + b)        [2 pairs x 128, 2048]  fp16
  vh       = x_v Wv_g^T          [2048, 4x65] fp16 (ones col -> row sums)
  scoresT  = khT^T qhT           per (pair, ktile, qtile), PSUM
  expT     = exp(scoresT/8)      ACT, fp16
  rawT     = vh_aug^T expT       PV matmul; row 64 = softmax denominator
  outT     = rawT[0:64] * (1/rawT[64])
  partialT = woT^T outT          [1024, 2048] fp32 -> DRAM
Host: out[b] = sum_g partialT(b,g)^T + (Wo bv + bo).

All matmul operands fp16 (PE is exact on fp16 inputs, fp32 accumulate);
softmax denominators come free via the ones column. No collectives.
"""

import numpy as np

B, S, D, H = 2, 2048, 1024, 16
DK = D // H          # 64
N_CORES = 8
HG = H // 4          # 4 head-groups
HL = 4               # heads per core
FEAT = HL * DK       # 256 per-core features
NQT = S // 512       # 4 query tiles
NKT = S // 128       # 16 key tiles
NDT = D // 128       # 8 contraction tiles (d-model)

_cache = {}


def _build():
    import concourse.mybir as mybir
    import concourse.tile as tile
    from concourse import bacc

    fp32 = mybir.dt.float32
    fp16 = mybir.dt.float16
    bf16 = mybir.dt.bfloat16

    nc = bacc.Bacc("TRN2", target_bir_lowering=False, debug=False,
                   num_devices=N_CORES)

    xqT = nc.dram_tensor("xqT", [D, S], fp16, kind="ExternalInput").ap()
    xkT = nc.dram_tensor("xkT", [D, S], fp16, kind="ExternalInput").ap()
    xvT = nc.dram_tensor("xvT", [D, S], fp16, kind="ExternalInput").ap()
    wqT = nc.dram_tensor("wqT", [D, FEAT], fp16, kind="ExternalInput").ap()
    wkT = nc.dram_tensor("wkT", [D, FEAT], fp16, kind="ExternalInput").ap()
    wvT = nc.dram_tensor("wvT", [D, FEAT], fp16, kind="ExternalInput").ap()
    woT = nc.dram_tensor("woT", [FEAT, D], fp16, kind="ExternalInput").ap()
    bq2 = nc.dram_tensor("bq2", [FEAT, 1], fp32, kind="ExternalInput").ap()
    bk2 = nc.dram_tensor("bk2", [FEAT, 1], fp32, kind="ExternalInput").ap()
    out_d = nc.dram_tensor("partialT", [D, S], fp32, kind="ExternalOutput").ap()

    with tile.TileContext(nc) as tc:
        with (
            tc.tile_pool(name="xin", bufs=1) as xin,
            tc.tile_pool(name="win", bufs=1) as win,
            tc.tile_pool(name="proj", bufs=1) as proj,
            tc.tile_pool(name="pexp", bufs=4) as pexp,
            tc.tile_pool(name="pout", bufs=4) as pout,
            tc.tile_pool(name="pnrm", bufs=4) as pnrm,
            tc.tile_pool(name="pp", bufs=2, space="PSUM") as pp,
            tc.tile_pool(name="ps2", bufs=2, space="PSUM") as ps2,
            tc.tile_pool(name="pspv", bufs=2, space="PSUM") as pspv,
        ):
            # ---- load inputs ----
            xq3 = xin.tile([128, NDT, S], fp16, tag="xq")
            xk3 = xin.tile([128, NDT, S], fp16, tag="xk")
            xv3 = xin.tile([128, NDT, S], fp16, tag="xv")
            nc.sync.dma_start(xq3[:], xqT.rearrange("(t p) s -> p t s", p=128))
            nc.sync.dma_start(xk3[:], xkT.rearrange("(t p) s -> p t s", p=128))
            nc.sync.dma_start(xv3[:], xvT.rearrange("(t p) s -> p t s", p=128))
            wq3 = win.tile([128, NDT, FEAT], fp16, tag="wq")
            wk3 = win.tile([128, NDT, FEAT], fp16, tag="wk")
            wv3 = win.tile([128, NDT, FEAT], fp16, tag="wv")
            nc.sync.dma_start(wq3[:], wqT.rearrange("(t p) f -> p t f", p=128))
            nc.sync.dma_start(wk3[:], wkT.rearrange("(t p) f -> p t f", p=128))
            nc.sync.dma_start(wv3[:], wvT.rearrange("(t p) f -> p t f", p=128))
            wo3 = win.tile([128, 2, D], fp16, tag="wo")
            nc.sync.dma_start(wo3[:], woT.rearrange("(t p) j -> p t j", p=128))
            bq3 = win.tile([128, 2, 1], fp32, tag="bq")
            bk3 = win.tile([128, 2, 1], fp32, tag="bk")
            nc.sync.dma_start(bq3[:], bq2.rearrange("(t p) o -> p t o", p=128))
            nc.sync.dma_start(bk3[:], bk2.rearrange("(t p) o -> p t o", p=128))

            # ---- persistent intermediates ----
            qh3 = proj.tile([128, 2, S], fp16, tag="qh")    # pair-packed q proj
            kh3 = proj.tile([128, 2, S], fp16, tag="kh")
            vha = proj.tile([128, NKT, HL, DK + 1], bf16, tag="vha")
            ot3 = proj.tile([128, 2, S], fp16, tag="outT")  # attention outT

            nc.gpsimd.memset(vha[:, :, :, DK], 1.0)  # ones column
            # bias tile for exp: -2 headroom under fp16 max (cancels in the
            # row-sum division)
            ebias = win.tile([128, 1], fp32, tag="ebias")
            nc.gpsimd.memset(ebias[:], -2.0)

            # ---- q/k projections: qh3[:, m, :] = (W x^T + b), fp16 ----
            for x3, w3, b3, dst in ((xq3, wq3, bq3, qh3), (xk3, wk3, bk3, kh3)):
                for m in range(2):           # feature tile = head pair
                    for n in range(NQT):
                        ps = pp.tile([128, 512], fp32, tag="acc")
                        for kt in range(NDT):
                            nc.tensor.matmul(
                                ps[:],
                                w3[:, kt, m * 128:(m + 1) * 128],
                                x3[:, kt, n * 512:(n + 1) * 512],
                                start=(kt == 0), stop=(kt == NDT - 1))
                        nc.vector.tensor_scalar_add(
                            dst[:, m, n * 512:(n + 1) * 512], ps[:], b3[:, m, :])

            # ---- v projection (no bias; folded into host-side Wo@bv) ----
            for st in range(NKT):
                ps = pp.tile([128, 256], fp32, tag="acc")
                for kt in range(NDT):
                    nc.tensor.matmul(
                        ps[:], xv3[:, kt, st * 128:(st + 1) * 128],
                        wv3[:, kt, :],
                        start=(kt == 0), stop=(kt == NDT - 1))
                nc.vector.tensor_copy(vha[:, st, :, 0:DK], ps[:])

            # ---- attention ----
            for qt in range(NQT):
                for hp in range(2):
                    pva = pspv.tile([DK + 1, 512], fp32, tag="pv")
                    pvb = pspv.tile([DK + 1, 512], fp32, tag="pv")
                    for kt in range(NKT):
                        s2 = ps2.tile([128, 1024], fp32, tag="s2")
                        nc.tensor.matmul(
                            s2[:, 0:512],
                            kh3[0:64, hp, kt * 128:(kt + 1) * 128],
                            qh3[0:64, hp, qt * 512:(qt + 1) * 512],
                            start=True, stop=True)
                        nc.tensor.matmul(
                            s2[:, 512:1024],
                            kh3[64:128, hp, kt * 128:(kt + 1) * 128],
                            qh3[64:128, hp, qt * 512:(qt + 1) * 512],
                            start=True, stop=True)
                        e2 = pexp.tile([128, 1024], bf16, tag="e2")
                        nc.scalar.activation(
                            e2[:], s2[:], mybir.ActivationFunctionType.Exp,
                            scale=0.125, bias=ebias[:])
                        nc.tensor.matmul(
                            pva[:], vha[:, kt, 2 * hp, :], e2[:, 0:512],
                            start=(kt == 0), stop=(kt == NKT - 1))
                        nc.tensor.matmul(
                            pvb[:], vha[:, kt, 2 * hp + 1, :], e2[:, 512:1024],
                            start=(kt == 0), stop=(kt == NKT - 1))
                    # normalize: outT[head] = pv[0:64] / pv[64]
                    for pv, half in ((pva, 0), (pvb, 1)):
                        # custom DVE ops must read SBUF, not PSUM
                        srow = pnrm.tile([1, 512], fp32, tag="srow")
                        nc.vector.tensor_copy(srow[:], pv[DK:DK + 1, :])
                        inv = pnrm.tile([1, 512], fp32, tag="inv")
                        nc.vector.reciprocal_approx_fast(inv[:], srow[:])
                        invb = pnrm.tile([64, 512], fp32, tag="invb")
                        nc.gpsimd.partition_broadcast(invb[:], inv[:])
                        nc.vector.tensor_tensor(
                            ot3[half * 64:(half + 1) * 64, hp,
                                qt * 512:(qt + 1) * 512],
                            pv[0:DK, :], invb[:], mybir.AluOpType.mult)

                # ---- output projection for this qtile ----
                for jt in range(NDT):
                    ps = pp.tile([128, 512], fp32, tag="acc")
                    for m in range(2):
                        nc.tensor.matmul(
                            ps[:], wo3[:, m, jt * 128:(jt + 1) * 128],
                            ot3[:, m, qt * 512:(qt + 1) * 512],
                            start=(m == 0), stop=(m == 1))
                    po = pout.tile([128, 512], fp32, tag="po")
                    nc.vector.tensor_copy(po[:], ps[:])
                    nc.sync.dma_start(
                        out_d[jt * 128:(jt + 1) * 128,
                              qt * 512:(qt + 1) * 512], po[:])

    nc.compile()
    return nc


def kernel(q, k, v, Wq, bq, Wk, bk, Wv, bv, Wo, bo, _trace=False):
    from concourse import bass_utils

    if "nc" not in _cache:
        _cache["nc"] = _build()
    nc = _cache["nc"]

    q = np.asarray(q, np.float32)
    k = np.asarray(k, np.float32)
    v = np.asarray(v, np.float32)
    Wq = np.asarray(Wq, np.float32)
    Wk = np.asarray(Wk, np.float32)
    Wv = np.asarray(Wv, np.float32)
    Wo = np.asarray(Wo, np.float32)
    bq = np.asarray(bq, np.float32)
    bk = np.asarray(bk, np.float32)
    bv = np.asarray(bv, np.float32)
    bo = np.asarray(bo, np.float32)

    xT = {}
    for b in range(B):
        xT[("q", b)] = np.ascontiguousarray(q[b].T).astype(np.float16)
        xT[("k", b)] = np.ascontiguousarray(k[b].T).astype(np.float16)
        xT[("v", b)] = np.ascontiguousarray(v[b].T).astype(np.float16)
    wT = {}
    for g in range(HG):
        sl = slice(g * FEAT, (g + 1) * FEAT)
        wT[("q", g)] = np.ascontiguousarray(Wq[sl, :].T).astype(np.float16)
        wT[("k", g)] = np.ascontiguousarray(Wk[sl, :].T).astype(np.float16)
        wT[("v", g)] = np.ascontiguousarray(Wv[sl, :].T).astype(np.float16)
        wT[("o", g)] = np.ascontiguousarray(Wo[:, sl].T).astype(np.float16)

    in_maps = []
    for c in range(N_CORES):
        b, g = divmod(c, HG)
        sl = slice(g * FEAT, (g + 1) * FEAT)
        in_maps.append({
            "xqT": xT[("q", b)], "xkT": xT[("k", b)], "xvT": xT[("v", b)],
            "wqT": wT[("q", g)], "wkT": wT[("k", g)], "wvT": wT[("v", g)],
            "woT": wT[("o", g)],
            "bq2": np.ascontiguousarray(bq[sl]).reshape(FEAT, 1),
            "bk2": np.ascontiguousarray(bk[sl]).reshape(FEAT, 1),
        })

    kwargs = {}
    if _trace:
        _install_profile_shim()
        kwargs = dict(trace=True, trace_cores=list(range(N_CORES)))
    res = bass_utils.run_bass_kernel_spmd(
        nc, in_maps, core_ids=list(range(N_CORES)), **kwargs)
    _cache["last_results"] = res

    final_bias = (Wo @ bv + bo).astype(np.float32)  # Wo@bv: attn rows sum to 1
    out = np.empty((B, S, D), np.float32)
    for b in range(B):
        acc = res.results[b * HG]["partialT"].copy()
        for g in range(1, HG):
            acc += res.results[b * HG + g]["partialT"]
        out[b] = acc.T + final_bias
    return out


def _install_profile_shim():
    """Provide antenv.axon_hooks so trace=True works under axon."""
    import sys
    import types

    import antenv

    if "antenv.axon_hooks" in sys.modules:
        return
    mod = types.ModuleType("antenv.axon_hooks")
    mod._hook = None
    mod.set_axon_ntff_profile_hook = lambda h: setattr(mod, "_hook", h)
    mod.get_axon_ntff_profile_hook = lambda: mod._hook
    sys.modules["antenv.axon_hooks"] = mod
    antenv.axon_hooks = mod
    try:
        from trn_agent_boot.trn_boot import _ntff_profile_via_ctypes
        mod.set_axon_ntff_profile_hook(
            _ntff_profile_via_ctypes("/opt/axon/libaxon_pjrt.so"))
    except Exception:
        pass
